# revision 18
# baseline (speedup 1.0000x reference)
"""Bass/Trainium2 kernel for nn_Decoder (attention GRU decoder, B=32 S=64 V=32000).

Strategy (8 NeuronCores, zero collectives):
  - teacher_forcing_ratio=1.0 and coins=uniform(key(42)) < 1 always => teacher
    forcing always wins, so per-step logits/argmax are NOT needed in the loop.
  - Data-parallel over batch: each core runs the 63-step attention+GRU
    recurrence for its 4 batch rows entirely locally (transposed layouts,
    d-on-partitions; fp16 stationary weights, fp32 PSUM accumulation).
  - The attention context never materializes: enc @ Wc.T is precomputed once
    (encW), and the per-step context contribution to the GRU input gates is a
    block-diagonal matmul with the softmax weights.
  - Final vocab projection is sharded over vocab (4000 cols/core, padded 4096)
    against the locally accumulated fp16 h-history.
"""

import numpy as np

B, S, V, EMB, E2, D = 32, 64, 32000, 512, 1024, 1024
T = S - 1          # 63 recurrent steps
NB = 4             # batch rows per core
NC = 8             # cores
J = 3 * D          # 3072 gate width
NDT = D // 128     # 8 d tiles
NJT = J // 128     # 24 j tiles
VS = V // NC       # 4000 vocab shard
VSP = 4096         # padded vocab shard

_CACHE = {}


def _tr(M):
    """[k*128, C] -> [128, k*C] with col = k_idx*C + c (tile rows for SBUF)."""
    R, C = M.shape
    k = R // 128
    return np.ascontiguousarray(M.reshape(k, 128, C).transpose(1, 0, 2).reshape(128, k * C))


def _f16(x):
    return np.ascontiguousarray(x, dtype=np.float16)


def _f32(x):
    return np.ascontiguousarray(x, dtype=np.float32)


def _build_program():
    import concourse.bass as bass
    import concourse.tile as tile
    from concourse import mybir

    f32, f16 = mybir.dt.float32, mybir.dt.float16
    AF = mybir.ActivationFunctionType
    ALU = mybir.AluOpType

    nc = bass.Bass()

    # ---- I/O ----
    encT_d = nc.dram_tensor('encT', [128, 8 * 256], f16, kind='ExternalInput')
    Wa_d = nc.dram_tensor('Wa', [128, 8 * 1024], f16, kind='ExternalInput')
    Ua_d = nc.dram_tensor('Ua', [128, 8 * 1024], f16, kind='ExternalInput')
    va_d = nc.dram_tensor('va', [128, 8], f16, kind='ExternalInput')
    WhhT_d = nc.dram_tensor('WhhT', [128, 8 * J], f16, kind='ExternalInput')
    WcTn_d = nc.dram_tensor('WcTn', [128, 6 * 8 * 512], f16, kind='ExternalInput')
    WeTaJ_d = nc.dram_tensor('WeTaJ', [128, NJT * 5 * 128], f16, kind='ExternalInput')
    embTa_d = nc.dram_tensor('embTa', [128, 5 * 256], f16, kind='ExternalInput')
    WoutT_d = nc.dram_tensor('WoutT', [128, 8 * VSP], f16, kind='ExternalInput')
    bout_d = nc.dram_tensor('bout', [1, VSP], f32, kind='ExternalInput')
    bhhn_d = nc.dram_tensor('bhhn4', [128, 32], f32, kind='ExternalInput')
    mask_d = nc.dram_tensor('maskr', [1, 256], f32, kind='ExternalInput')
    bdm_d = nc.dram_tensor('bdmask', [128, 8], f32, kind='ExternalInput')
    h0T_d = nc.dram_tensor('h0T', [128, 32], f32, kind='ExternalInput')
    OUT_d = nc.dram_tensor('OUT', [2048, VSP], f32, kind='ExternalOutput')

    import os
    KSTEPS = int(os.environ.get('KSTEPS', str(T)))
    KDEBUG = os.environ.get('KDEBUG', '') == '1'
    if KDEBUG:
        DBG1_d = nc.dram_tensor('DBG1', [128, 200], f32, kind='ExternalOutput')
        DBG2_d = nc.dram_tensor('DBG2', [1, 256], f32, kind='ExternalOutput')
        DBG3_d = nc.dram_tensor('DBG3', [128, 512], f32, kind='ExternalOutput')
        DBG4_d = nc.dram_tensor('DBG4', [128, 256], f32, kind='ExternalOutput')

    with tile.TileContext(nc) as tc:
        from contextlib import ExitStack
        with ExitStack() as ctx:
            persist = ctx.enter_context(tc.tile_pool(name='persist', bufs=1))
            hT = persist.tile([128, 32], f32)
            HT16 = persist.tile([128, 8 * 256], f16)
            h016 = persist.tile([128, 32], f16)
            va_sb = persist.tile([128, 8], f16)
            mask_sb = persist.tile([1, 256], f32)
            bdm_sb = persist.tile([128, 8], f32)
            bhhn_sb = persist.tile([128, 32], f32)
            ones1 = persist.tile([1, 1], f32)
            onesr = persist.tile([1, 128], f32)

            nc.gpsimd.dma_start(hT[:], h0T_d[:])
            nc.gpsimd.dma_start(va_sb[:], va_d[:])
            nc.gpsimd.dma_start(mask_sb[:], mask_d[:])
            nc.gpsimd.dma_start(bdm_sb[:], bdm_d[:])
            nc.gpsimd.dma_start(bhhn_sb[:], bhhn_d[:])
            nc.vector.memset(ones1[:], 1.0)
            nc.vector.memset(onesr[:], 1.0)
            nc.vector.tensor_copy(h016[:], hT[:])
            nc.vector.memset(HT16[:], 0.0)

            rec_stack = ctx.enter_context(ExitStack())
            recres = rec_stack.enter_context(tc.tile_pool(name='recres', bufs=1))
            Ua_sb = recres.tile([128, 8 * 1024], f16)
            Whh_sb = recres.tile([128, 8 * J], f16)
            encW = recres.tile([128, 2 * J], f16)
            embW = recres.tile([128, NJT * 256], f16)
            encproj = recres.tile([128, 8 * 256], f32)
            nc.gpsimd.dma_start(Ua_sb[:], Ua_d[:])
            nc.gpsimd.dma_start(Whh_sb[:], WhhT_d[:])

            # ---- precompute A: embW = WeTa.T @ embTa  ([j, (t,b)]) ----
            with tc.tile_pool(name='preA', bufs=1) as preA, \
                 tc.tile_pool(name='preAps', bufs=2, space='PSUM') as preAps:
                WeTaJ_sb = preA.tile([128, NJT * 5 * 128], f16)
                embTa_sb = preA.tile([128, 5 * 256], f16)
                nc.gpsimd.dma_start(WeTaJ_sb[:], WeTaJ_d[:])
                nc.gpsimd.dma_start(embTa_sb[:], embTa_d[:])
                for jt in range(NJT):
                    ps = preAps.tile([128, 256], f32, tag='psA')
                    for q in range(5):
                        nc.tensor.matmul(
                            ps[:],
                            WeTaJ_sb[:, (jt * 5 + q) * 128:(jt * 5 + q + 1) * 128],
                            embTa_sb[:, q * 256:(q + 1) * 256],
                            start=(q == 0), stop=(q == 4))
                    nc.scalar.copy(embW[:, jt * 256:(jt + 1) * 256], ps[:])

            # ---- precompute B: enc_proj = Wa.T @ encT ; encW = encT.T @ WcT ----
            with tc.tile_pool(name='preB', bufs=1) as preB, \
                 tc.tile_pool(name='preBps', bufs=2, space='PSUM') as preBps:
                encT_sb = preB.tile([128, 8 * 256], f16)
                Wa_sb = preB.tile([128, 8 * 1024], f16)
                WcTn_sb = preB.tile([128, 6 * 8 * 512], f16)
                nc.gpsimd.dma_start(encT_sb[:], encT_d[:])
                nc.gpsimd.dma_start(Wa_sb[:], Wa_d[:])
                nc.gpsimd.dma_start(WcTn_sb[:], WcTn_d[:])
                for dt in range(8):
                    ps = preBps.tile([128, 256], f32, tag='psB1')
                    for et in range(8):
                        nc.tensor.matmul(
                            ps[:],
                            Wa_sb[:, et * 1024 + dt * 128: et * 1024 + (dt + 1) * 128],
                            encT_sb[:, et * 256:(et + 1) * 256],
                            start=(et == 0), stop=(et == 7))
                    nc.scalar.copy(encproj[:, dt * 256:(dt + 1) * 256], ps[:])
                for kt in range(2):
                    for n in range(6):
                        ps = preBps.tile([128, 512], f32, tag='psB2')
                        for et in range(8):
                            nc.tensor.matmul(
                                ps[:],
                                encT_sb[:, et * 256 + kt * 128: et * 256 + (kt + 1) * 128],
                                WcTn_sb[:, n * 4096 + et * 512: n * 4096 + (et + 1) * 512],
                                start=(et == 0), stop=(et == 7))
                        nc.scalar.copy(encW[:, kt * J + n * 512: kt * J + (n + 1) * 512], ps[:])

            # ---- recurrent loop ----
            lt = rec_stack.enter_context(tc.tile_pool(name='lt', bufs=2))
            lps = rec_stack.enter_context(tc.tile_pool(name='lps', bufs=1, space='PSUM'))
            if True:
                for t in range(KSTEPS):
                    if t == 0:
                        def hsrc(k):
                            return h016[:, k * 4:(k + 1) * 4]
                    else:
                        toff = (t - 1) * 4

                        def hsrc(k, toff=toff):
                            return HT16[:, k * 256 + toff: k * 256 + toff + 4]

                    # hUa (transposed) and gh (transposed)
                    hua_ps = lps.tile([128, 32], f32, tag='hua')
                    for dt in range(8):
                        for et in range(8):
                            nc.tensor.matmul(
                                hua_ps[:, dt * 4:(dt + 1) * 4],
                                Ua_sb[:, et * 1024 + dt * 128: et * 1024 + (dt + 1) * 128],
                                hsrc(et),
                                start=(et == 0), stop=(et == 7))
                    gh_ps = lps.tile([128, 96], f32, tag='gh')
                    for jt in range(NJT):
                        for kt in range(8):
                            nc.tensor.matmul(
                                gh_ps[:, jt * 4:(jt + 1) * 4],
                                Whh_sb[:, kt * J + jt * 128: kt * J + (jt + 1) * 128],
                                hsrc(kt),
                                start=(kt == 0), stop=(kt == 7))

                    hua_sb = lt.tile([128, 32], f32, tag='huasb')
                    nc.vector.tensor_copy(hua_sb[:], hua_ps[:])

                    # energy = tanh(encproj + hUa[b]) , fp16
                    energy = lt.tile([128, 8 * 256], f16, tag='energy')
                    for dt in range(8):
                        for b in range(NB):
                            o = dt * 256 + b * 64
                            nc.scalar.activation(
                                energy[:, o:o + 64], encproj[:, o:o + 64],
                                AF.Tanh, bias=hua_sb[:, dt * 4 + b: dt * 4 + b + 1])

                    # scores = va . energy  -> [1, 256]
                    sc_ps = lps.tile([1, 256], f32, tag='sc')
                    for dt in range(8):
                        nc.tensor.matmul(
                            sc_ps[:], va_sb[:, dt:dt + 1],
                            energy[:, dt * 256:(dt + 1) * 256],
                            start=(dt == 0), stop=(dt == 7))

                    smask = lt.tile([1, 256], f32, tag='smask')
                    nc.vector.tensor_tensor(out=smask[:], in0=sc_ps[:], in1=mask_sb[:], op=ALU.add)
                    wexp = lt.tile([1, 256], f32, tag='wexp')
                    nc.scalar.activation(wexp[:], smask[:], AF.Exp)
                    sums = lt.tile([1, 4], f32, tag='sums')
                    nc.vector.tensor_reduce(
                        sums[:], wexp[0:1, :].rearrange("p (b s) -> p b s", s=64),
                        axis=mybir.AxisListType.X, op=ALU.add)
                    recip = lt.tile([1, 4], f32, tag='recip')
                    nc.vector.reciprocal(recip[:], sums[:])

                    # transpose wexp -> [256, 1] via K=1 matmuls; recip -> 128 partitions
                    wt_ps = lps.tile([128, 2], f32, tag='wt')
                    for kt in range(2):
                        nc.tensor.matmul(
                            wt_ps[:, kt:kt + 1], wexp[0:1, kt * 128:(kt + 1) * 128],
                            ones1[0:1, 0:1], start=True, stop=True)
                    bc_ps = lps.tile([128, 4], f32, tag='bc')
                    nc.tensor.matmul(bc_ps[:], onesr[0:1, :], recip[0:1, :], start=True, stop=True)
                    wT_sb = lt.tile([128, 2], f32, tag='wTsb')
                    nc.vector.tensor_copy(wT_sb[:], wt_ps[:])

                    wbd32 = lt.tile([128, 8], f32, tag='wbd32')
                    Wbd16 = lt.tile([128, 8], f16, tag='wbd16')
                    for kt in range(2):
                        nc.vector.tensor_scalar_mul(
                            wbd32[:, kt * 4:(kt + 1) * 4], bdm_sb[:, kt * 4:(kt + 1) * 4],
                            wT_sb[:, kt:kt + 1])
                        nc.vector.tensor_tensor(
                            out=Wbd16[:, kt * 4:(kt + 1) * 4],
                            in0=wbd32[:, kt * 4:(kt + 1) * 4], in1=bc_ps[:], op=ALU.mult)

                    # gxc (transposed): encW.T @ Wbd (own psum bank)
                    gxc_ps = lps.tile([128, 96], f32, tag='gxc')
                    for jt in range(NJT):
                        for kt in range(2):
                            nc.tensor.matmul(
                                gxc_ps[:, jt * 4:(jt + 1) * 4],
                                encW[:, kt * J + jt * 128: kt * J + (jt + 1) * 128],
                                Wbd16[:, kt * 4:(kt + 1) * 4],
                                start=(kt == 0), stop=(kt == 1))

                    # gates; all [128, 32] viewed [128, 8, 4]
                    def v84(ap):
                        return ap.rearrange("p (a c) -> p a c", c=4)
                    embw_g = embW[:, :].rearrange("p (j c) -> p j c", c=256)
                    tsl = slice(t * 4, t * 4 + 4)

                    s_r = lt.tile([128, 32], f32, tag='s_r')
                    nc.vector.tensor_tensor(out=v84(s_r[:, :]), in0=v84(gh_ps[:, 0:32]), in1=embw_g[:, 0:8, tsl], op=ALU.add)
                    tmp_r2 = lt.tile([128, 32], f32, tag='tmp_r2')
                    nc.vector.tensor_tensor(out=tmp_r2[:], in0=s_r[:], in1=gxc_ps[:, 0:32], op=ALU.add)
                    r_sb = lt.tile([128, 32], f32, tag='r_sb')
                    nc.scalar.activation(r_sb[:], tmp_r2[:], AF.Sigmoid)

                    s_z = lt.tile([128, 32], f32, tag='s_z')
                    nc.vector.tensor_tensor(out=v84(s_z[:, :]), in0=v84(gh_ps[:, 32:64]), in1=embw_g[:, 8:16, tsl], op=ALU.add)
                    tmp_z2 = lt.tile([128, 32], f32, tag='tmp_z2')
                    nc.vector.tensor_tensor(out=tmp_z2[:], in0=s_z[:], in1=gxc_ps[:, 32:64], op=ALU.add)
                    z_sb = lt.tile([128, 32], f32, tag='z_sb')
                    nc.scalar.activation(z_sb[:], tmp_z2[:], AF.Sigmoid)

                    ghn = lt.tile([128, 32], f32, tag='ghn')
                    nc.vector.tensor_tensor(out=ghn[:], in0=gh_ps[:, 64:96], in1=bhhn_sb[:], op=ALU.add)
                    a1 = lt.tile([128, 32], f32, tag='a1')
                    nc.vector.tensor_tensor(out=v84(a1[:, :]), in0=v84(gxc_ps[:, 64:96]), in1=embw_g[:, 16:24, tsl], op=ALU.add)
                    m1 = lt.tile([128, 32], f32, tag='m1')
                    nc.vector.tensor_tensor(out=m1[:], in0=r_sb[:], in1=ghn[:], op=ALU.mult)
                    a2 = lt.tile([128, 32], f32, tag='a2')
                    nc.vector.tensor_tensor(out=a2[:], in0=a1[:], in1=m1[:], op=ALU.add)
                    n_sb = lt.tile([128, 32], f32, tag='n_sb')
                    nc.scalar.activation(n_sb[:], a2[:], AF.Tanh)

                    s1 = lt.tile([128, 32], f32, tag='s1')
                    nc.vector.tensor_tensor(out=s1[:], in0=hT[:], in1=n_sb[:], op=ALU.subtract)
                    m2 = lt.tile([128, 32], f32, tag='m2')
                    nc.vector.tensor_tensor(out=m2[:], in0=z_sb[:], in1=s1[:], op=ALU.mult)
                    nc.vector.tensor_tensor(out=hT[:], in0=n_sb[:], in1=m2[:], op=ALU.add)

                    # store h_t (fp16) into history
                    HT_view = HT16[:, :].rearrange("p (k c) -> p k c", c=256)
                    nc.vector.tensor_copy(HT_view[:, :, tsl], v84(hT[:, :]))

                    if KDEBUG and t == 0:
                        dbg = lt.tile([128, 200], f32, tag='dbg')
                        nc.vector.tensor_copy(dbg[:, 0:32], hua_sb[:])
                        nc.vector.tensor_copy(dbg[:, 32:64], hT[:])
                        nc.vector.tensor_copy(dbg[:, 64:72], Wbd16[:])
                        nc.vector.tensor_copy(dbg[:, 72:104], gh_ps[:, 0:32])
                        nc.vector.tensor_copy(dbg[:, 104:136], ghn[:])
                        nc.vector.tensor_copy(dbg[:, 136:168], gxc_ps[:, 64:96])
                        nc.vector.tensor_copy(
                            v84(dbg[:, 168:200]), embw_g[:, 0:8, tsl])
                        nc.gpsimd.dma_start(DBG1_d[:], dbg[:])
                        nc.gpsimd.dma_start(DBG2_d[:], wexp[:])
                        dbg3 = lt.tile([128, 512], f32, tag='dbg3')
                        nc.vector.tensor_copy(dbg3[:, 0:256], encproj[:, 0:256])
                        nc.vector.tensor_copy(dbg3[:, 256:512], energy[:, 0:256])
                        nc.gpsimd.dma_start(DBG3_d[:], dbg3[:])

            # ---- all-gather h history across the 8 cores ----
            rec_stack.close()
            with tc.tile_pool(name='agd', bufs=1, space='DRAM') as agd, \
                 tc.tile_pool(name='lg', bufs=1) as lg, \
                 tc.tile_pool(name='ev', bufs=2) as ev, \
                 tc.tile_pool(name='lgps', bufs=2, space='PSUM') as lgps:
                hag_in = agd.tile([128, 8 * 256], f16)
                hag_out = agd.tile([8 * 128, 8 * 256], f16)
                nc.gpsimd.dma_start(hag_in[:], HT16[:])
                nc.gpsimd.collective_compute(
                    "AllGather", ALU.bypass,
                    replica_groups=[list(range(NC))],
                    ins=[hag_in.opt()], outs=[hag_out.opt()])
                # HA[p, k*2048 + r*256 + t*4 + bl] = h[d=k*128+p, t, b=4r+bl]
                HA = lg.tile([128, 8 * 2048], f16)
                for k in range(8):
                    src = hag_out[:, k * 256:(k + 1) * 256].rearrange(
                        "(r p) c -> p r c", p=128)
                    dst = HA[:, k * 2048:(k + 1) * 2048].rearrange(
                        "p (r c) -> p r c", c=256)
                    nc.gpsimd.dma_start(dst, src)

                Wout_sb = lg.tile([128, 8 * VSP], f16)
                bout_sb = lg.tile([1, VSP], f32)
                nc.gpsimd.dma_start(Wout_sb[:], WoutT_d[:])
                nc.gpsimd.dma_start(bout_sb[:], bout_d[:])

                if KDEBUG:
                    dbg4 = ev.tile([128, 256], f32, tag='dbg4')
                    nc.vector.tensor_copy(dbg4[:], HT16[:, 0:256])
                    nc.gpsimd.dma_start(DBG4_d[:], dbg4[:])

                # OUT[(r,t,bl), v] = HA.T @ WoutT (+ bout); rows m*128
                for m in range(16):
                    for half in range(2):
                        ps = lgps.tile([128, 2048], f32, tag='lgps')
                        for n in range(4):
                            for k in range(8):
                                nc.tensor.matmul(
                                    ps[:, n * 512:(n + 1) * 512],
                                    HA[:, k * 2048 + m * 128: k * 2048 + (m + 1) * 128],
                                    Wout_sb[:, k * VSP + half * 2048 + n * 512: k * VSP + half * 2048 + (n + 1) * 512],
                                    start=(k == 0), stop=False)
                            nc.tensor.matmul(
                                ps[:, n * 512:(n + 1) * 512],
                                onesr[0:1, :],
                                bout_sb[0:1, half * 2048 + n * 512: half * 2048 + (n + 1) * 512],
                                start=False, stop=True)
                        evt = ev.tile([128, 2048], f32, tag='ev')
                        nc.vector.tensor_copy(evt[:], ps[:])
                        nc.gpsimd.dma_start(
                            OUT_d[m * 128:(m + 1) * 128, half * 2048:(half + 1) * 2048], evt[:])

    _split_excess_waits(nc, max_waits=1)
    return nc


def _split_excess_waits(nc, max_waits=1):
    """This walrus build rejects instructions with more than one sync wait;
    move excess on_wait entries onto same-engine NoOps inserted just before."""
    import concourse.mybir as mybir
    ctr = 0
    for f in nc.m.functions:
        for bb in f.blocks:
            out = []
            changed = False
            for inst in bb.instructions:
                si = inst.sync_info
                if si is not None and si.on_wait and len(si.on_wait) > max_waits:
                    waits = list(si.on_wait)
                    extra, keep = waits[:-max_waits], waits[-max_waits:]
                    for i in range(0, len(extra), max_waits):
                        ctr += 1
                        out.append(mybir.InstNoOp(
                            name=f"I-waitsplit-{ctr}", engine=inst.engine,
                            ins=[], outs=[],
                            sync_info=mybir.SyncInfo(
                                on_wait=list(extra[i:i + max_waits]), on_update=[]),
                            bass_nofuse=True))
                    inst.sync_info = mybir.SyncInfo(on_wait=keep, on_update=list(si.on_update))
                    changed = True
                out.append(inst)
            if changed:
                try:
                    bb.instructions[:] = out
                except Exception:
                    bb.instructions = out
    return ctr


def _coins():
    import jax
    cpu = jax.devices('cpu')[0]
    with jax.default_device(cpu):
        return np.asarray(jax.random.uniform(jax.random.key(42), (T,)))


def _fallback(tokens, hidden0, encoder_outs, lengths, tfr, embedding, Wa, Ua, v_a,
              W_ih, b_ih, W_hh, b_hh, W_out, b_out, coins):
    """Pure-numpy replica of the reference (handles the greedy/argmax path)."""
    Bv, Sv = encoder_outs.shape[0], encoder_outs.shape[1]
    Vv = embedding.shape[0]
    mask = np.arange(Sv)[None, :] < lengths[:, None]
    enc_proj = np.einsum('bse,ed->bsd', encoder_outs, Wa)
    teach = embedding[tokens]
    h = hidden0.astype(np.float32)
    prev = np.zeros((Bv, Vv), np.float32)
    prev[:, 0] = 1.0
    outs = [prev.copy()]
    for t in range(Sv - 1):
        energy = np.tanh(enc_proj + (h @ Ua)[:, None, :])
        scores = np.einsum('bsd,d->bs', energy, v_a)
        scores = np.where(mask, scores, -np.inf)
        e = np.exp(scores - scores.max(1, keepdims=True))
        w = e / e.sum(1, keepdims=True)
        ctx = np.einsum('bs,bse->be', w, encoder_outs)
        sel = teach[:, t] if coins[t] < tfr else embedding[prev.argmax(1)]
        x = np.concatenate([sel, ctx], 1)
        gx = x @ W_ih.T + b_ih
        gh = h @ W_hh.T + b_hh
        gxr, gxz, gxn = np.split(gx, 3, 1)
        ghr, ghz, ghn = np.split(gh, 3, 1)
        r = 1 / (1 + np.exp(-(gxr + ghr)))
        z = 1 / (1 + np.exp(-(gxz + ghz)))
        n = np.tanh(gxn + r * ghn)
        h = (1 - z) * n + z * h
        prev = (h @ W_out.T + b_out).astype(np.float32)
        outs.append(prev.copy())
    return np.stack(outs, 1)


def kernel(tokens, hidden0, encoder_outs, lengths, teacher_forcing_ratio,
           embedding, Wa, Ua, v_a, W_ih, b_ih, W_hh, b_hh, W_out, b_out):
    tokens = np.asarray(tokens)
    hidden0 = np.asarray(hidden0, np.float32)
    encoder_outs = np.asarray(encoder_outs, np.float32)
    lengths = np.asarray(lengths)
    tfr = float(np.asarray(teacher_forcing_ratio))
    embedding = np.asarray(embedding, np.float32)
    Wa = np.asarray(Wa, np.float32); Ua = np.asarray(Ua, np.float32)
    v_a = np.asarray(v_a, np.float32)
    W_ih = np.asarray(W_ih, np.float32); b_ih = np.asarray(b_ih, np.float32)
    W_hh = np.asarray(W_hh, np.float32); b_hh = np.asarray(b_hh, np.float32)
    W_out = np.asarray(W_out, np.float32); b_out = np.asarray(b_out, np.float32)

    coins = _coins()
    if not np.all(coins < tfr):
        return _fallback(tokens, hidden0, encoder_outs, lengths, tfr, embedding,
                         Wa, Ua, v_a, W_ih, b_ih, W_hh, b_hh, W_out, b_out, coins)

    from concourse.bass_utils import run_bass_kernel_spmd

    if 'nc' not in _CACHE:
        _CACHE['nc'] = _build_program()
    nc = _CACHE['nc']

    We, Wc = W_ih[:, :EMB], W_ih[:, EMB:]
    bfold = b_ih.copy()
    bfold[:2 * D] += b_hh[:2 * D]            # b_hh for n-gate stays separate
    teach = embedding[tokens]                # [B, 63, 512]

    # core-independent weight layouts
    Wa_l = _f16(_tr(Wa))
    Ua_l = _f16(_tr(Ua))
    va_l = _f16(v_a.reshape(8, 128).T)
    WhhT_l = _f16(_tr(np.ascontiguousarray(W_hh.T)))
    WcT = np.ascontiguousarray(Wc.T)         # [1024, 3072]
    WcTn_l = _f16(WcT.reshape(8, 128, 6, 512).transpose(1, 2, 0, 3).reshape(128, 6 * 8 * 512))
    WeTa = np.zeros((640, J), np.float32)
    WeTa[:EMB] = We.T
    WeTa[EMB] = bfold
    WeTaJ_l = _f16(WeTa.reshape(5, 128, NJT, 128).transpose(1, 2, 0, 3).reshape(128, NJT * 5 * 128))
    bhhn_l = _f32(np.repeat(b_hh[2 * D:].reshape(8, 128).T[:, :, None], 4, axis=2).reshape(128, 32))
    bdm = np.zeros((128, 8), np.float32)
    for kt in range(2):
        for p in range(128):
            bdm[p, kt * 4 + (kt * 128 + p) // 64] = 1.0

    in_maps = []
    for c in range(NC):
        sl = slice(c * NB, (c + 1) * NB)
        enc_c = encoder_outs[sl]                                   # [4, 64, 1024]
        encT_l = _f16(enc_c.transpose(2, 0, 1).reshape(8, 128, NB * S)
                      .transpose(1, 0, 2).reshape(128, 8 * 256))
        # note: transpose(2,0,1) gives [e, b, s]; col must be b*64+s (b-major) - yes
        embTa = np.zeros((640, 256), np.float32)
        embTa[:EMB, :T * NB] = teach[sl].transpose(2, 1, 0).reshape(EMB, T * NB)
        embTa[EMB, :T * NB] = 1.0
        embTa_l = _f16(embTa.reshape(5, 128, 256).transpose(1, 0, 2).reshape(128, 5 * 256))
        Wp = np.zeros((D, VSP), np.float32)
        Wp[:, :VS] = W_out[c * VS:(c + 1) * VS].T
        WoutT_l = _f16(_tr(Wp))
        bout_l = np.zeros((1, VSP), np.float32)
        bout_l[0, :VS] = b_out[c * VS:(c + 1) * VS]
        maskr = np.where(np.arange(S)[None, :] < lengths[sl][:, None], 0.0, -60.0)
        maskr_l = _f32(maskr.reshape(1, 256))
        h0T_l = _f32(hidden0[sl].T.reshape(8, 128, NB).transpose(1, 0, 2).reshape(128, 32))
        in_maps.append(dict(
            encT=encT_l, Wa=Wa_l, Ua=Ua_l, va=va_l, WhhT=WhhT_l, WcTn=WcTn_l,
            WeTaJ=WeTaJ_l, embTa=embTa_l, WoutT=WoutT_l, bout=bout_l,
            bhhn4=bhhn_l, maskr=maskr_l, bdmask=bdm, h0T=h0T_l))

    res = run_bass_kernel_spmd(nc, in_maps, core_ids=list(range(NC)))
    _CACHE['last_result'] = res

    out = np.zeros((B, S, V), np.float32)
    out[:, 0, 0] = 1.0
    for c in range(NC):
        # OUT rows = r*256 + t*4 + bl  (b_global = 4r + bl), vocab shard c
        r = res.results[c]['OUT'].reshape(NC, S, NB, VSP)[:, :T, :, :VS]
        out[:, 1:, c * VS:(c + 1) * VS] = r.transpose(0, 2, 1, 3).reshape(B, T, VS)
    return out
